# revision 1
# baseline (speedup 1.0000x reference)
"""Trainium2 Bass kernel for nn_MHA_48120813584614 (dual cross-attention MHA).

Strategy (head/tensor parallel over 8 cores):
  - Core c owns head c for BOTH attention directions:
      pair 0 ("i"): metadata queries attend image keys/values  -> contributes to out_i
      pair 1 ("m"): image queries attend metadata keys/values -> contributes to out_m
  - All tensors kept transposed ([feature, token]) so no transposes are needed:
      QT[e,m] = WqT.T @ XTq ; KT[e,n] = WkT.T @ XTkv ; V[n,e] = XTkv.T @ WvT
      ST[n,m] = KT.T @ QT   (scores transposed: keys on partitions)
      E = exp(ST/sqrt(512)) (no max subtraction; logits are O(1) here)
      colsum[m] = ones.T @ E (softmax denominator via PE)
      OT[e,m]  = V.T @ E     (unnormalized attention output)
      partialT[o,m] = WlT.T @ OT ; then scaled by 1/colsum[m] (normalization
      commutes with the head-output linear)
  - Row-parallel output linear: per-head partials are ReduceScattered over the
    8 cores (one RS per direction; dir-i RS overlaps dir-m compute). Each core
    gets a 64-row shard of the transposed output, adds b_lin slice + residual
    slice, and returns it. Host concatenates + transposes.
  - All matmuls bf16 (fp32 PSUM accumulation); softmax denominator fp32.
"""

import sys

sys.path.insert(0, "/opt/trn_rl_repo")

import math

import ml_dtypes
import numpy as np

import concourse.bass as bass
import concourse.mybir as mybir
import concourse.tile as tile
from concourse import bacc
from concourse.bass_utils import run_bass_kernel_spmd

H = 8
D = 512
N = 2048
NCORES = 8
P = 128
MC = 512  # m-chunk (matmul free dim / PSUM bank)
NMC = N // MC  # 4
ET = D // P  # 4 e/d/o tiles
NT = N // P  # 16 n tiles
SCALE = 1.0 / math.sqrt(D)

bf16 = mybir.dt.bfloat16
f32 = mybir.dt.float32

AF = mybir.ActivationFunctionType


def _build(reps=1, single=False, no_cc=False):
    ndev = 1 if single else NCORES
    nc = bacc.Bacc("TRN2", target_bir_lowering=False, debug=False, num_devices=ndev)

    def din(name, shape, dtype):
        return nc.dram_tensor(name, shape, dtype, kind="ExternalInput").ap()

    xt = [din("xt_i", [D, N], bf16), din("xt_m", [D, N], bf16)]
    wq = [din(f"wq{p}", [D, D], bf16) for p in range(2)]
    wk = [din(f"wk{p}", [D, D], bf16) for p in range(2)]
    wv = [din(f"wv{p}", [D, D], bf16) for p in range(2)]
    wl = [din(f"wl{p}", [D, D], bf16) for p in range(2)]
    bq = [din(f"bq{p}", [P, ET], f32) for p in range(2)]
    bk = [din(f"bk{p}", [P, ET], f32) for p in range(2)]
    bv = [din(f"bv{p}", [1, D], f32) for p in range(2)]
    ones128_d = din("ones128", [P, 1], bf16)
    resid = [din("resid_i", [64, N], f32), din("resid_m", [64, N], f32)]
    out_d = [
        nc.dram_tensor("out_i", [64, N], f32, kind="ExternalOutput").ap(),
        nc.dram_tensor("out_m", [64, N], f32, kind="ExternalOutput").ap(),
    ]

    with tile.TileContext(nc) as tc:
        with (
            tc.tile_pool(name="const", bufs=1) as cpool,
            tc.tile_pool(name="xt", bufs=1) as xtpool,
            tc.tile_pool(name="w", bufs=2) as wpool,
            tc.tile_pool(name="qkv", bufs=1) as qkvpool,
            tc.tile_pool(name="v", bufs=1) as vpool,
            tc.tile_pool(name="expst", bufs=2) as epool,
            tc.tile_pool(name="ot", bufs=2) as otpool,
            tc.tile_pool(name="small", bufs=2) as spool,
            tc.tile_pool(name="outsb", bufs=2) as opool,
            tc.tile_pool(name="post", bufs=1) as ppool,
            tc.tile_pool(name="psum", bufs=6, space="PSUM") as ps,
            tc.tile_pool(name="pscs", bufs=2, space="PSUM") as pscs,
            tc.tile_pool(name="dram", bufs=1, space="DRAM") as dr,
        ):
          for _rep in range(reps):
            # ---- constants / inputs ----
            xts = [xtpool.tile([P, ET, N], bf16, tag=f"xt{i}", name=f"xt{i}")
                   for i in range(2)]
            for i in (1, 0):  # pair 0's QT pass reads xt_m: load it fully first
                srcq = xt[i].rearrange("(t p) n -> p t n", p=P)
                for dt_ in range(ET):
                    nc.sync.dma_start(xts[i][:, dt_, :], srcq[:, dt_, :])
            ones128 = cpool.tile([P, 1], bf16)
            nc.sync.dma_start(ones128[:], ones128_d[:])
            resid_sb = []
            for i in range(2):
                rt = cpool.tile([64, N], f32, tag=f"resid{i}")
                nc.sync.dma_start(rt[:], resid[i][:])
                resid_sb.append(rt)

            rs_in = [
                dr.tile([D, N], bf16, tag=f"rsin{p}", name=f"rsin{p}")
                for p in range(2)
            ]
            rs_out = [
                dr.tile([64, N], bf16, tag=f"rsout{p}", name=f"rsout{p}")
                for p in range(2)
            ]

            for p in range(2):
                xq = xts[1] if p == 0 else xts[0]  # query-side input (transposed)
                xkv = xts[0] if p == 0 else xts[1]  # key/value-side input

                # ---- per-pair weights ----
                wq_t = wpool.tile([P, ET, D], bf16, tag="wq")
                wk_t = wpool.tile([P, ET, D], bf16, tag="wk")
                wv_t = wpool.tile([P, ET, D], bf16, tag="wv")
                wl_t = wpool.tile([P, ET, D], bf16, tag="wl")
                for w_t, w_d in ((wq_t, wq[p]), (wk_t, wk[p]), (wv_t, wv[p]),
                                 (wl_t, wl[p])):
                    src = w_d.rearrange("(t p) e -> p t e", p=P)
                    for dt_ in range(ET):
                        nc.sync.dma_start(w_t[:, dt_, :], src[:, dt_, :])
                bq_t = wpool.tile([P, ET], f32, tag="bq")
                nc.sync.dma_start(bq_t[:], bq[p][:])
                bk_t = wpool.tile([P, ET], f32, tag="bk")
                nc.sync.dma_start(bk_t[:], bk[p][:])
                bv_t = wpool.tile([1, D], f32, tag="bv")
                nc.sync.dma_start(bv_t[:], bv[p][:])

                # ---- projections: QT/KT [e,tile][m], V [n,tile][e] ----
                qt_t = qkvpool.tile([P, ET, N], bf16, tag="qt")
                kt_t = qkvpool.tile([P, ET, N], bf16, tag="kt")
                v_t = vpool.tile([P, NT, D], bf16, tag="v")
                for w_p, x_p, dst_p, b_p in (
                    (wq_t, xq, qt_t, bq_t),
                    (wk_t, xkv, kt_t, bk_t),
                ):
                    for eb in range(ET):
                        for mc in range(NMC):
                            psq = ps.tile([P, MC], f32, tag="ps", name="psq")
                            for dt_ in range(ET):
                                nc.tensor.matmul(
                                    psq[:],
                                    w_p[:, dt_, eb * P:(eb + 1) * P],
                                    x_p[:, dt_, mc * MC:(mc + 1) * MC],
                                    start=(dt_ == 0),
                                    stop=(dt_ == ET - 1),
                                )
                            nc.scalar.activation(
                                dst_p[:, eb, mc * MC:(mc + 1) * MC], psq[:],
                                AF.Identity, bias=b_p[:, eb:eb + 1],
                            )
                bv_bc = wpool.tile([P, D], f32, tag="bvbc")
                nc.gpsimd.partition_broadcast(bv_bc[:], bv_t[:])
                for nt in range(NT):
                    psv = ps.tile([P, D], f32, tag="ps")
                    for dt_ in range(ET):
                        nc.tensor.matmul(
                            psv[:],
                            xkv[:, dt_, nt * P:(nt + 1) * P],
                            wv_t[:, dt_, :],
                            start=(dt_ == 0),
                            stop=(dt_ == ET - 1),
                        )
                    # bias add (broadcast along partitions) fused into the copy
                    nc.vector.tensor_tensor(
                        v_t[:, nt, :], psv[:], bv_bc[:], mybir.AluOpType.add
                    )

                # ---- attention + output linear, per m-chunk ----
                for mc in range(NMC):
                    e_t = epool.tile([P, NT, MC], bf16, tag="e")
                    acc = spool.tile([P, MC], f32, tag="acc")
                    for nt in range(NT):
                        pss = ps.tile([P, MC], f32, tag="ps")
                        for eb in range(ET):
                            nc.tensor.matmul(
                                pss[:],
                                kt_t[:, eb, nt * P:(nt + 1) * P],
                                qt_t[:, eb, mc * MC:(mc + 1) * MC],
                                start=(eb == 0),
                                stop=(eb == ET - 1),
                            )
                        nc.scalar.activation(e_t[:, nt, :], pss[:], AF.Exp, scale=SCALE)
                        # running per-partition sum of exp tiles (DVE)
                        if nt == 0:
                            nc.vector.tensor_copy(acc[:], e_t[:, 0, :])
                        else:
                            nc.vector.tensor_tensor(
                                acc[:], acc[:], e_t[:, nt, :], mybir.AluOpType.add
                            )
                    # softmax denominator: reduce acc over partitions (1 bf16 MM)
                    acc_bf = spool.tile([P, MC], bf16, tag="accbf")
                    nc.vector.tensor_copy(acc_bf[:], acc[:])
                    cs = pscs.tile([1, MC], f32, tag="cs")
                    nc.tensor.matmul(cs[:], ones128[:], acc_bf[:], start=True, stop=True)
                    recip = spool.tile([1, MC], f32, tag="recip")
                    nc.vector.reciprocal(recip[:], cs[:])
                    rb = spool.tile([P, MC], f32, tag="rb", name="rb")
                    nc.gpsimd.partition_broadcast(rb[:], recip[:])
                    # PV: OT[e, m] unnormalized
                    ot_t = otpool.tile([P, ET, MC], bf16, tag="ot")
                    for eb in range(ET):
                        pso = ps.tile([P, MC], f32, tag="ps")
                        for nt in range(NT):
                            nc.tensor.matmul(
                                pso[:],
                                v_t[:, nt, eb * P:(eb + 1) * P],
                                e_t[:, nt, :],
                                start=(nt == 0),
                                stop=(nt == NT - 1),
                            )
                        nc.vector.tensor_copy(ot_t[:, eb, :], pso[:])
                    # output linear partial + deferred softmax normalization
                    for ob in range(ET):
                        psl = ps.tile([P, MC], f32, tag="ps")
                        for eb in range(ET):
                            nc.tensor.matmul(
                                psl[:],
                                wl_t[:, eb, ob * P:(ob + 1) * P],
                                ot_t[:, eb, :],
                                start=(eb == 0),
                                stop=(eb == ET - 1),
                            )
                        res_sb = opool.tile([P, MC], bf16, tag="res")
                        nc.vector.tensor_tensor(
                            res_sb[:], psl[:], rb[:], mybir.AluOpType.mult
                        )
                        nc.sync.dma_start(
                            rs_in[p][ob * P:(ob + 1) * P, mc * MC:(mc + 1) * MC],
                            res_sb[:],
                        )

                # ---- reduce-scatter partials over all 8 cores (bf16) ----
                if not single and not no_cc:
                    nc.gpsimd.collective_compute(
                        "ReduceScatter",
                        mybir.AluOpType.add,
                        ins=[rs_in[p].opt()],
                        outs=[rs_out[p].opt()],
                        replica_groups=[list(range(NCORES))],
                    )
                po_bf = ppool.tile([64, N], bf16, tag="pobf")
                nc.sync.dma_start(
                    po_bf[:],
                    rs_out[p][:] if not (single or no_cc) else rs_in[p][0:64, :],
                )
                po = ppool.tile([64, N], f32, tag="po")
                nc.vector.tensor_tensor(
                    po[:], po_bf[:], resid_sb[p][:], mybir.AluOpType.add
                )
                nc.sync.dma_start(out_d[p][:], po[:])

    nc.compile()
    return nc


_NC_CACHE = {}


def _get_nc():
    if "nc" not in _NC_CACHE:
        _NC_CACHE["nc"] = _build()
    return _NC_CACHE["nc"]


def _make_in_maps(inputs):
    f = np.float32
    b = ml_dtypes.bfloat16

    def c_(x, dt):
        return np.ascontiguousarray(x).astype(dt)

    img = np.asarray(inputs["image_input"], f)
    meta = np.asarray(inputs["metadata_input"], f)
    xt_i = c_(img.T, b)
    xt_m = c_(meta.T, b)
    ones128 = np.ones((P, 1), b)

    in_maps = []
    for c in range(NCORES):
        m = {
            "xt_i": xt_i,
            "xt_m": xt_m,
            "ones128": ones128,
            "resid_i": c_(img[:, 64 * c:64 * (c + 1)].T
                          + np.asarray(inputs["b_lin_i"], f)[64 * c:64 * (c + 1)][:, None], f),
            "resid_m": c_(meta[:, 64 * c:64 * (c + 1)].T
                          + np.asarray(inputs["b_lin_m"], f)[64 * c:64 * (c + 1)][:, None], f),
        }
        for p, (Wq, bq_, Wk, bk_, Wv, bv_, Wl) in enumerate([
            (inputs["Wq_m"], inputs["bq_m"], inputs["Wk_i"], inputs["bk_i"],
             inputs["Wv_i"], inputs["bv_i"], inputs["W_lin_i"]),
            (inputs["Wq_i"], inputs["bq_i"], inputs["Wk_m"], inputs["bk_m"],
             inputs["Wv_m"], inputs["bv_m"], inputs["W_lin_m"]),
        ]):
            m[f"wq{p}"] = c_(np.asarray(Wq, f)[c].T, b)
            m[f"wk{p}"] = c_(np.asarray(Wk, f)[c].T, b)
            m[f"wv{p}"] = c_(np.asarray(Wv, f)[c].T, b)
            m[f"wl{p}"] = c_(np.asarray(Wl, f)[:, D * c:D * (c + 1)].T, b)
            m[f"bq{p}"] = c_(np.asarray(bq_, f)[c].reshape(ET, P).T, f)
            m[f"bk{p}"] = c_(np.asarray(bk_, f)[c].reshape(ET, P).T, f)
            m[f"bv{p}"] = c_(np.asarray(bv_, f)[c][None, :], f)
        in_maps.append(m)
    return in_maps


def _assemble(results):
    out_iT = np.concatenate([results[c]["out_i"] for c in range(NCORES)], axis=0)
    out_mT = np.concatenate([results[c]["out_m"] for c in range(NCORES)], axis=0)
    return np.concatenate([out_iT.T, out_mT.T], axis=1).astype(np.float32)


def kernel(**inputs):
    nc = _get_nc()
    in_maps = _make_in_maps(inputs)
    res = run_bass_kernel_spmd(nc, in_maps, list(range(NCORES)))
    return _assemble(res.results)


if __name__ == "__main__":
    _get_nc()
    print("build ok")



# revision 25
# speedup vs baseline: 6.8190x; 6.8190x over previous
"""Trainium2 Bass kernel for nn_MHA_48120813584614 (dual cross-attention MHA).

Strategy (head/tensor parallel over 8 cores), fp8 DoubleRow matmuls:
  - Core c owns head c for BOTH attention directions:
      pair 0 ("i"): metadata queries attend image keys/values  -> contributes to out_i
      pair 1 ("m"): image queries attend metadata keys/values -> contributes to out_m
  - All tensors kept transposed ([feature, token]) so no transposes are needed:
      QT[e,m] = WqT.T @ XTq ; KT[e,n] = WkT.T @ XTkv ; V[n,e] = XTkv.T @ WvT
      ST[n,m] = KT.T @ QT   (scores transposed: keys on partitions)
      E = exp(ST*scale)     (no max subtraction; logits are O(1) here)
      colsum[m] = ones.T @ E (softmax denominator via PE, fp8 DoubleRow)
      OT[e,m]  = V.T @ E     (unnormalized attention output)
      partialT[o,m] = WlT.T @ OT ; then scaled by 1/colsum[m] (normalization
      commutes with the head-output linear)
  - All matmuls are fp8e4m3 with DoubleRow perf mode (2 contraction planes
    per instruction, 2x bf16 throughput). Host pre-scales weights into the
    fp8 normal range: Wq,Wk,Wv x8, Wl x16; the colsum "ones" vector is 128
    (=8*16) so the reciprocal of colsum absorbs every scale exactly.
  - The v-bias is folded into the host-side residual (att rows sum to 1:
    res = att@v0 + bv, so out += W_l @ bv, a constant).
  - Row-parallel output linear: per-head partials are ReduceScattered over the
    8 cores (one RS per direction; dir-i RS overlaps dir-m compute). Each core
    gets a 64-row shard of the transposed output, adds (b_lin + W_l@bv +
    residual) slice, and returns it. Host concatenates + transposes.
"""

import sys

sys.path.insert(0, "/opt/trn_rl_repo")

import math

import ml_dtypes
import numpy as np

import concourse.bass as bass
import concourse.mybir as mybir
import concourse.tile as tile
from concourse import bacc
from concourse.bass_utils import run_bass_kernel_spmd

H = 8
D = 512
N = 2048
NCORES = 8
P = 128
MC = 512  # m-chunk (matmul free dim / PSUM bank)
NMC = N // MC  # 4
ET = D // P  # 4 e/d/o tiles
NT = N // P  # 16 n tiles

SQ = 8.0  # host scale on Wq, bq
SK = 8.0  # host scale on Wk, bk
SV = 0.5  # host scale on Wv (OT = E@V over 2048 keys must stay inside fp8 max 240)
SL = 16.0  # host scale on W_lin
ONESVAL = SL * SV  # colsum ones value: recip(colsum*SL*SV) cancels SL*SV in psl
ESCALE = 1.0 / (math.sqrt(D) * SQ * SK)

bf16 = mybir.dt.bfloat16
f32 = mybir.dt.float32
f8 = mybir.dt.float8e4

AF = mybir.ActivationFunctionType
DR = mybir.MatmulPerfMode.DoubleRow


def _build(reps=1, single=False, no_cc=False):
    ndev = 1 if single else NCORES
    nc = bacc.Bacc("TRN2", target_bir_lowering=False, debug=False, num_devices=ndev)

    def din(name, shape, dtype):
        return nc.dram_tensor(name, shape, dtype, kind="ExternalInput").ap()

    xt = [din("xt_i", [D, N], f8), din("xt_m", [D, N], f8)]
    wq = [din(f"wq{p}", [D, D], f8) for p in range(2)]
    wk = [din(f"wk{p}", [D, D], f8) for p in range(2)]
    wv = [din(f"wv{p}", [D, D], f8) for p in range(2)]
    wl = [din(f"wl{p}", [D, D], f8) for p in range(2)]
    bq = [din(f"bq{p}", [P, ET], f32) for p in range(2)]
    bk = [din(f"bk{p}", [P, ET], f32) for p in range(2)]
    ones2_d = din("ones2", [P, 2 * P], f8)
    resid = [din("resid_i", [64, N], f32), din("resid_m", [64, N], f32)]
    out_d = [
        nc.dram_tensor("out_i", [64, N], f32, kind="ExternalOutput").ap(),
        nc.dram_tensor("out_m", [64, N], f32, kind="ExternalOutput").ap(),
    ]

    pools = {}

    def _emit_rep():
        cpool, xtpool, wpool, qkvpool, vpool, epool, otpool, spool, opool, \
            ppool, ps, ps2, pscs, dr = (
                pools[k] for k in ("const", "xt", "w", "qkv", "v", "expst",
                                   "ot", "small", "outsb", "post", "psum",
                                   "psum2", "pscs", "dram"))
        # ---- constants / inputs (single batched DMA per tensor; biases
        # before the big weight loads so the first activations never wait) ----
        xts = [xtpool.tile([P, ET, N], f8, tag=f"xt{i}", name=f"xt{i}")
               for i in range(2)]
        for i in (1, 0):  # pair 0's QT pass reads xt_m: load it fully first
            nc.sync.dma_start(xts[i][:], xt[i].rearrange("(t p) n -> p t n", p=P))
        ones2 = cpool.tile([P, 2, P], f8)
        nc.sync.dma_start(ones2[:], ones2_d.rearrange("p (k c) -> p k c", k=2))
        bq_ts, bk_ts = [], []
        for p in range(2):
            bq_t = wpool.tile([P, ET], f32, tag=f"bq{p}")
            nc.sync.dma_start(bq_t[:], bq[p][:])
            bk_t = wpool.tile([P, ET], f32, tag=f"bk{p}")
            nc.sync.dma_start(bk_t[:], bk[p][:])
            bq_ts.append(bq_t)
            bk_ts.append(bk_t)
        wts = []
        for p in range(2):
            wq_t = wpool.tile([P, ET, D], f8, tag=f"wq{p}")
            wk_t = wpool.tile([P, ET, D], f8, tag=f"wk{p}")
            wv_t = wpool.tile([P, ET, D], f8, tag=f"wv{p}")
            wl_t = wpool.tile([P, ET, D], f8, tag=f"wl{p}")
            for w_t, w_d in ((wq_t, wq[p]), (wk_t, wk[p]), (wv_t, wv[p]),
                             (wl_t, wl[p])):
                nc.sync.dma_start(w_t[:], w_d.rearrange("(t p) e -> p t e", p=P))
            wts.append((wq_t, wk_t, wv_t, wl_t))
        resid_sb = []
        for i in range(2):
            rt = cpool.tile([64, N], f32, tag=f"resid{i}")
            nc.sync.dma_start(rt[:], resid[i][:])
            resid_sb.append(rt)

        # per-(direction, m-chunk) RS slabs: the collective for chunk mc
        # overlaps the next chunk's compute; only the last slab's RS is a tail
        rs_in = [
            [dr.tile([D, MC], bf16, tag=f"rsin{p}_{c}", name=f"rsin{p}_{c}")
             for c in range(NMC)]
            for p in range(2)
        ]
        rs_out = [
            [dr.tile([64, MC], bf16, tag=f"rsout{p}_{c}", name=f"rsout{p}_{c}")
             for c in range(NMC)]
            for p in range(2)
        ]

        for p in range(2):
            xq = xts[1] if p == 0 else xts[0]  # query-side input (transposed)
            xkv = xts[0] if p == 0 else xts[1]  # key/value-side input

            # ---- per-pair weights (loaded at rep start) ----
            wq_t, wk_t, wv_t, wl_t = wts[p]
            bq_t, bk_t = bq_ts[p], bk_ts[p]

            # ---- projections: QT/KT [e,tile][m], V [n,tile][e] ----
            qt_t = qkvpool.tile([P, ET, N], f8, tag="qt")
            kt_t = qkvpool.tile([P, ET, N], f8, tag="kt")
            v_t = vpool.tile([P, NT, D], f8, tag="v")
            for w_p, x_p, dst_p, b_p in (
                (wq_t, xq, qt_t, bq_t),
                (wk_t, xkv, kt_t, bk_t),
            ):
                for eb in range(ET):
                    for mh in range(NMC // 2):
                        psq = ps2.tile([P, 2, MC], f32, tag="ps2", name="psq")
                        for sub in range(2):
                            mc = 2 * mh + sub
                            for dh in range(ET // 2):
                                nc.tensor.matmul(
                                    psq[:, sub, :],
                                    w_p[:, 2 * dh:2 * dh + 2, eb * P:(eb + 1) * P],
                                    x_p[:, 2 * dh:2 * dh + 2, mc * MC:(mc + 1) * MC],
                                    start=(dh == 0),
                                    stop=(dh == ET // 2 - 1),
                                    perf_mode=DR,
                                )
                        nc.scalar.activation(
                            dst_p[:, eb, 2 * mh * MC:(2 * mh + 2) * MC], psq[:],
                            AF.Identity, bias=b_p[:, eb:eb + 1],
                        )
            for nh in range(NT // 2):
                psv = ps2.tile([P, 2, D], f32, tag="ps2", name="psv")
                for sub in range(2):
                    nt = 2 * nh + sub
                    for dh in range(ET // 2):
                        nc.tensor.matmul(
                            psv[:, sub, :],
                            xkv[:, 2 * dh:2 * dh + 2, nt * P:(nt + 1) * P],
                            wv_t[:, 2 * dh:2 * dh + 2, :],
                            start=(dh == 0),
                            stop=(dh == ET // 2 - 1),
                            perf_mode=DR,
                        )
                nc.vector.tensor_copy(v_t[:, 2 * nh:2 * nh + 2, :], psv[:])

            # ---- attention + output linear, per m-chunk ----
            for mc in range(NMC):
                e_t = epool.tile([P, NT, MC], f8, tag="e")
                for nh in range(NT // 2):
                    pss = ps2.tile([P, 2, MC], f32, tag="ps2", name="pss")
                    for sub in range(2):
                        nt = 2 * nh + sub
                        for eh in range(ET // 2):
                            nc.tensor.matmul(
                                pss[:, sub, :],
                                kt_t[:, 2 * eh:2 * eh + 2, nt * P:(nt + 1) * P],
                                qt_t[:, 2 * eh:2 * eh + 2, mc * MC:(mc + 1) * MC],
                                start=(eh == 0),
                                stop=(eh == ET // 2 - 1),
                                perf_mode=DR,
                            )
                    nc.scalar.activation(e_t[:, 2 * nh:2 * nh + 2, :], pss[:],
                                         AF.Exp, scale=ESCALE)
                # softmax denominator via PE: ones(=SL*SV) dot E columns,
                # broadcast across all 128 partitions so the reciprocal
                # directly yields the normalization tile rb
                cs = pscs.tile([P, MC], f32, tag="cs")
                for nh in range(NT // 2):
                    nc.tensor.matmul(
                        cs[:], ones2[:],
                        e_t[:, 2 * nh:2 * nh + 2, :],
                        start=(nh == 0), stop=(nh == NT // 2 - 1),
                        perf_mode=DR,
                    )
                rb = spool.tile([P, MC], f32, tag="rb", name="rb")
                nc.vector.reciprocal(rb[:], cs[:])
                # PV: OT[e, m] unnormalized (single-bank psum so the next
                # m-chunk's score pairs in ps2 are not blocked behind PV)
                ot_t = otpool.tile([P, ET, MC], f8, tag="ot")
                for eb in range(ET):
                    pso = ps.tile([P, MC], f32, tag="ps", name="pso")
                    for nh in range(NT // 2):
                        nc.tensor.matmul(
                            pso[:],
                            v_t[:, 2 * nh:2 * nh + 2, eb * P:(eb + 1) * P],
                            e_t[:, 2 * nh:2 * nh + 2, :],
                            start=(nh == 0),
                            stop=(nh == NT // 2 - 1),
                            perf_mode=DR,
                        )
                    nc.vector.tensor_copy(ot_t[:, eb, :], pso[:])
                # output linear partial + deferred softmax normalization;
                # all 4 ob blocks collect into one SBUF tile -> single DMA
                res_sb = opool.tile([P, ET, MC], bf16, tag="res")
                for ob in range(ET):
                    psl = ps.tile([P, MC], f32, tag="ps")
                    for eh in range(ET // 2):
                        nc.tensor.matmul(
                            psl[:],
                            wl_t[:, 2 * eh:2 * eh + 2, ob * P:(ob + 1) * P],
                            ot_t[:, 2 * eh:2 * eh + 2, :],
                            start=(eh == 0),
                            stop=(eh == ET // 2 - 1),
                            perf_mode=DR,
                        )
                    nc.vector.tensor_tensor(
                        res_sb[:, ob, :], psl[:], rb[:], mybir.AluOpType.mult
                    )
                nc.sync.dma_start(
                    rs_in[p][mc].rearrange("(o q) m -> q o m", q=P),
                    res_sb[:],
                )

                # ---- reduce-scatter this chunk's partials over the 8 cores;
                # overlaps the next chunk's compute ----
                if not single and not no_cc:
                    nc.gpsimd.collective_compute(
                        "ReduceScatter",
                        mybir.AluOpType.add,
                        ins=[rs_in[p][mc].opt()],
                        outs=[rs_out[p][mc].opt()],
                        replica_groups=[list(range(NCORES))],
                    )
                po_bf = ppool.tile([64, MC], bf16, tag="pobf")
                nc.sync.dma_start(
                    po_bf[:],
                    rs_out[p][mc][:] if not (single or no_cc)
                    else rs_in[p][mc][0:64, :],
                )
                po = ppool.tile([64, MC], f32, tag="po")
                nc.gpsimd.tensor_tensor(
                    po[:], po_bf[:], resid_sb[p][:, mc * MC:(mc + 1) * MC],
                    mybir.AluOpType.add,
                )
                nc.sync.dma_start(out_d[p][:, mc * MC:(mc + 1) * MC], po[:])

    with tile.TileContext(nc) as tc:
        with (
            tc.tile_pool(name="const", bufs=1) as pools["const"],
            tc.tile_pool(name="xt", bufs=1) as pools["xt"],
            tc.tile_pool(name="w", bufs=2) as pools["w"],
            tc.tile_pool(name="qkv", bufs=1) as pools["qkv"],
            tc.tile_pool(name="v", bufs=1) as pools["v"],
            tc.tile_pool(name="expst", bufs=2) as pools["expst"],
            tc.tile_pool(name="ot", bufs=2) as pools["ot"],
            tc.tile_pool(name="small", bufs=2) as pools["small"],
            tc.tile_pool(name="outsb", bufs=2) as pools["outsb"],
            tc.tile_pool(name="post", bufs=2) as pools["post"],
            tc.tile_pool(name="psum", bufs=3, space="PSUM") as pools["psum"],
            tc.tile_pool(name="psum2", bufs=2, space="PSUM") as pools["psum2"],
            tc.tile_pool(name="pscs", bufs=1, space="PSUM") as pools["pscs"],
            tc.tile_pool(name="dram", bufs=1, space="DRAM") as pools["dram"],
        ):
            for _rep in range(reps):
                _emit_rep()

    nc.compile()
    return nc


_NC_CACHE = {}


def _get_nc():
    if "nc" not in _NC_CACHE:
        _NC_CACHE["nc"] = _build()
    return _NC_CACHE["nc"]


def _make_in_maps(inputs):
    f = np.float32
    b8 = ml_dtypes.float8_e4m3

    def c_(x, dt):
        return np.ascontiguousarray(x).astype(dt)

    img = np.asarray(inputs["image_input"], f)
    meta = np.asarray(inputs["metadata_input"], f)
    xt_i = c_(img.T, b8)
    xt_m = c_(meta.T, b8)
    ones2 = np.full((P, 2 * P), ONESVAL, b8)

    # fold v-bias through the output linear into the residual (att rows sum
    # to 1): out += W_lin @ concat_h(bv_h)
    W_li = np.asarray(inputs["W_lin_i"], f)
    W_lm = np.asarray(inputs["W_lin_m"], f)
    corr_i = W_li @ np.asarray(inputs["bv_i"], f).reshape(H * D)
    corr_m = W_lm @ np.asarray(inputs["bv_m"], f).reshape(H * D)
    base_i = np.asarray(inputs["b_lin_i"], f) + corr_i
    base_m = np.asarray(inputs["b_lin_m"], f) + corr_m

    in_maps = []
    for c in range(NCORES):
        sl = slice(64 * c, 64 * (c + 1))
        m = {
            "xt_i": xt_i,
            "xt_m": xt_m,
            "ones2": ones2,
            "resid_i": c_(img[:, sl].T + base_i[sl][:, None], f),
            "resid_m": c_(meta[:, sl].T + base_m[sl][:, None], f),
        }
        for p, (Wq, bq_, Wk, bk_, Wv, Wl) in enumerate([
            (inputs["Wq_m"], inputs["bq_m"], inputs["Wk_i"], inputs["bk_i"],
             inputs["Wv_i"], W_li),
            (inputs["Wq_i"], inputs["bq_i"], inputs["Wk_m"], inputs["bk_m"],
             inputs["Wv_m"], W_lm),
        ]):
            m[f"wq{p}"] = c_(np.asarray(Wq, f)[c].T * SQ, b8)
            m[f"wk{p}"] = c_(np.asarray(Wk, f)[c].T * SK, b8)
            m[f"wv{p}"] = c_(np.asarray(Wv, f)[c].T * SV, b8)
            m[f"wl{p}"] = c_(np.asarray(Wl, f)[:, D * c:D * (c + 1)].T * SL, b8)
            m[f"bq{p}"] = c_(np.asarray(bq_, f)[c].reshape(ET, P).T * SQ, f)
            m[f"bk{p}"] = c_(np.asarray(bk_, f)[c].reshape(ET, P).T * SK, f)
        in_maps.append(m)
    return in_maps


def _assemble(results):
    out_iT = np.concatenate([results[c]["out_i"] for c in range(NCORES)], axis=0)
    out_mT = np.concatenate([results[c]["out_m"] for c in range(NCORES)], axis=0)
    return np.concatenate([out_iT.T, out_mT.T], axis=1).astype(np.float32)


def kernel(**inputs):
    nc = _get_nc()
    in_maps = _make_in_maps(inputs)
    res = run_bass_kernel_spmd(nc, in_maps, list(range(NCORES)))
    return _assemble(res.results)


if __name__ == "__main__":
    _get_nc()
    print("build ok")


# revision 28
# speedup vs baseline: 6.9610x; 1.0208x over previous
"""Trainium2 Bass kernel for nn_MHA_48120813584614 (dual cross-attention MHA).

Strategy (head/tensor parallel over 8 cores), fp8 DoubleRow matmuls:
  - Core c owns head c for BOTH attention directions:
      pair 0 ("i"): metadata queries attend image keys/values  -> contributes to out_i
      pair 1 ("m"): image queries attend metadata keys/values -> contributes to out_m
  - All tensors kept transposed ([feature, token]) so no transposes are needed:
      QT[e,m] = WqT.T @ XTq ; KT[e,n] = WkT.T @ XTkv ; V[n,e] = XTkv.T @ WvT
      ST[n,m] = KT.T @ QT   (scores transposed: keys on partitions)
      E = exp(ST*scale)     (no max subtraction; logits are O(1) here)
      colsum[m] = ones.T @ E (softmax denominator via PE, fp8 DoubleRow)
      OT[e,m]  = V.T @ E     (unnormalized attention output)
      partialT[o,m] = WlT.T @ OT ; then scaled by 1/colsum[m] (normalization
      commutes with the head-output linear)
  - All matmuls are fp8e4m3 with DoubleRow perf mode (2 contraction planes
    per instruction, 2x bf16 throughput). Host pre-scales weights into the
    fp8 normal range: Wq,Wk,Wv x8, Wl x16; the colsum "ones" vector is 128
    (=8*16) so the reciprocal of colsum absorbs every scale exactly.
  - The v-bias is folded into the host-side residual (att rows sum to 1:
    res = att@v0 + bv, so out += W_l @ bv, a constant).
  - Row-parallel output linear: per-head partials are ReduceScattered over the
    8 cores (one RS per direction; dir-i RS overlaps dir-m compute). Each core
    gets a 64-row shard of the transposed output, adds (b_lin + W_l@bv +
    residual) slice, and returns it. Host concatenates + transposes.
"""

import sys

sys.path.insert(0, "/opt/trn_rl_repo")

import math

import ml_dtypes
import numpy as np

import concourse.bass as bass
import concourse.mybir as mybir
import concourse.tile as tile
from concourse import bacc
from concourse.bass_utils import run_bass_kernel_spmd

H = 8
D = 512
N = 2048
NCORES = 8
P = 128
MC = 512  # m-chunk (matmul free dim / PSUM bank)
NMC = N // MC  # 4
ET = D // P  # 4 e/d/o tiles
NT = N // P  # 16 n tiles

SQ = 8.0  # host scale on Wq, bq
SK = 8.0  # host scale on Wk, bk
SV = 0.5  # host scale on Wv (OT = E@V over 2048 keys must stay inside fp8 max 240)
SL = 16.0  # host scale on W_lin
ONESVAL = SL * SV  # colsum ones value: recip(colsum*SL*SV) cancels SL*SV in psl
ESCALE = 1.0 / (math.sqrt(D) * SQ * SK)

bf16 = mybir.dt.bfloat16
f32 = mybir.dt.float32
f8 = mybir.dt.float8e4

AF = mybir.ActivationFunctionType
DR = mybir.MatmulPerfMode.DoubleRow


def _build(reps=1, single=False, no_cc=False):
    ndev = 1 if single else NCORES
    nc = bacc.Bacc("TRN2", target_bir_lowering=False, debug=False, num_devices=ndev)

    def din(name, shape, dtype):
        return nc.dram_tensor(name, shape, dtype, kind="ExternalInput").ap()

    xt = [din("xt_i", [D, N], f8), din("xt_m", [D, N], f8)]
    wq = [din(f"wq{p}", [D, D], f8) for p in range(2)]
    wk = [din(f"wk{p}", [D, D], f8) for p in range(2)]
    wv = [din(f"wv{p}", [D, D], f8) for p in range(2)]
    wl = [din(f"wl{p}", [D, D], f8) for p in range(2)]
    bq = [din(f"bq{p}", [P, ET], f32) for p in range(2)]
    bk = [din(f"bk{p}", [P, ET], f32) for p in range(2)]
    ones2_d = din("ones2", [P, 2 * P], f8)
    resid = [din("resid_i", [64, N], f32), din("resid_m", [64, N], f32)]
    out_d = [
        nc.dram_tensor("out_i", [64, N], f32, kind="ExternalOutput").ap(),
        nc.dram_tensor("out_m", [64, N], f32, kind="ExternalOutput").ap(),
    ]

    pools = {}

    def _emit_rep():
        cpool, xtpool, wpool, qkvpool, vpool, epool, otpool, spool, opool, \
            ppool, ps, ps2, pscs, dr = (
                pools[k] for k in ("const", "xt", "w", "qkv", "v", "expst",
                                   "ot", "small", "outsb", "post", "psum",
                                   "psum2", "pscs", "dram"))
        # ---- constants / inputs (single batched DMA per tensor; biases
        # before the big weight loads so the first activations never wait) ----
        xts = [xtpool.tile([P, ET, N], f8, tag=f"xt{i}", name=f"xt{i}")
               for i in range(2)]
        for i in (1, 0):  # pair 0's QT pass reads xt_m: load it fully first
            nc.sync.dma_start(xts[i][:], xt[i].rearrange("(t p) n -> p t n", p=P))
        ones2 = cpool.tile([P, 2, P], f8)
        nc.sync.dma_start(ones2[:], ones2_d.rearrange("p (k c) -> p k c", k=2))
        bq_ts, bk_ts = [], []
        for p in range(2):
            bq_t = wpool.tile([P, ET], f32, tag=f"bq{p}")
            nc.sync.dma_start(bq_t[:], bq[p][:])
            bk_t = wpool.tile([P, ET], f32, tag=f"bk{p}")
            nc.sync.dma_start(bk_t[:], bk[p][:])
            bq_ts.append(bq_t)
            bk_ts.append(bk_t)
        wts = []
        for p in range(2):
            wq_t = wpool.tile([P, ET, D], f8, tag=f"wq{p}")
            wk_t = wpool.tile([P, ET, D], f8, tag=f"wk{p}")
            wv_t = wpool.tile([P, ET, D], f8, tag=f"wv{p}")
            wl_t = wpool.tile([P, ET, D], f8, tag=f"wl{p}")
            for w_t, w_d in ((wq_t, wq[p]), (wk_t, wk[p]), (wv_t, wv[p]),
                             (wl_t, wl[p])):
                nc.sync.dma_start(w_t[:], w_d.rearrange("(t p) e -> p t e", p=P))
            wts.append((wq_t, wk_t, wv_t, wl_t))
        resid_sb = []
        for i in range(2):
            rt = cpool.tile([64, N], f32, tag=f"resid{i}")
            nc.sync.dma_start(rt[:], resid[i][:])
            resid_sb.append(rt)

        # per-(direction, m-chunk) RS slabs: the collective for chunk mc
        # overlaps the next chunk's compute; only the last slab's RS is a tail
        rs_in = [
            [dr.tile([D, MC], bf16, tag=f"rsin{p}_{c}", name=f"rsin{p}_{c}")
             for c in range(NMC)]
            for p in range(2)
        ]
        rs_out = [
            [dr.tile([64, MC], bf16, tag=f"rsout{p}_{c}", name=f"rsout{p}_{c}")
             for c in range(NMC)]
            for p in range(2)
        ]

        for p in range(2):
            xq = xts[1] if p == 0 else xts[0]  # query-side input (transposed)
            xkv = xts[0] if p == 0 else xts[1]  # key/value-side input

            # ---- per-pair weights (loaded at rep start) ----
            wq_t, wk_t, wv_t, wl_t = wts[p]
            bq_t, bk_t = bq_ts[p], bk_ts[p]

            # ---- projections: QT/KT [e,tile][m], V [n,tile][e] ----
            qt_t = qkvpool.tile([P, ET, N], f8, tag="qt")
            kt_t = qkvpool.tile([P, ET, N], f8, tag="kt")
            v_t = vpool.tile([P, NT, D], f8, tag="v")
            for w_p, x_p, dst_p, b_p in (
                (wq_t, xq, qt_t, bq_t),
                (wk_t, xkv, kt_t, bk_t),
            ):
                for eb in range(ET):
                    for mh in range(NMC // 2):
                        psq = ps2.tile([P, 2, MC], f32, tag="ps2", name="psq")
                        for sub in range(2):
                            mc = 2 * mh + sub
                            for dh in range(ET // 2):
                                nc.tensor.matmul(
                                    psq[:, sub, :],
                                    w_p[:, 2 * dh:2 * dh + 2, eb * P:(eb + 1) * P],
                                    x_p[:, 2 * dh:2 * dh + 2, mc * MC:(mc + 1) * MC],
                                    start=(dh == 0),
                                    stop=(dh == ET // 2 - 1),
                                    perf_mode=DR,
                                )
                        nc.scalar.activation(
                            dst_p[:, eb, 2 * mh * MC:(2 * mh + 2) * MC], psq[:],
                            AF.Identity, bias=b_p[:, eb:eb + 1],
                        )
            for nh in range(NT // 2):
                psv = ps2.tile([P, 2, D], f32, tag="ps2", name="psv")
                for sub in range(2):
                    nt = 2 * nh + sub
                    for dh in range(ET // 2):
                        nc.tensor.matmul(
                            psv[:, sub, :],
                            xkv[:, 2 * dh:2 * dh + 2, nt * P:(nt + 1) * P],
                            wv_t[:, 2 * dh:2 * dh + 2, :],
                            start=(dh == 0),
                            stop=(dh == ET // 2 - 1),
                            perf_mode=DR,
                        )
                nc.vector.tensor_copy(v_t[:, 2 * nh:2 * nh + 2, :], psv[:])

            # ---- attention + output linear, per m-chunk ----
            for mc in range(NMC):
                e_t = epool.tile([P, NT, MC], f8, tag="e")
                for nh in range(NT // 2):
                    pss = ps2.tile([P, 2, MC], f32, tag="ps2", name="pss")
                    for sub in range(2):
                        nt = 2 * nh + sub
                        for eh in range(ET // 2):
                            nc.tensor.matmul(
                                pss[:, sub, :],
                                kt_t[:, 2 * eh:2 * eh + 2, nt * P:(nt + 1) * P],
                                qt_t[:, 2 * eh:2 * eh + 2, mc * MC:(mc + 1) * MC],
                                start=(eh == 0),
                                stop=(eh == ET // 2 - 1),
                                perf_mode=DR,
                            )
                    nc.scalar.activation(e_t[:, 2 * nh:2 * nh + 2, :], pss[:],
                                         AF.Exp, scale=ESCALE)
                # softmax denominator via PE: ones(=SL*SV) dot E columns,
                # broadcast across all 128 partitions so the reciprocal
                # directly yields the normalization tile rb
                cs = pscs.tile([P, MC], f32, tag="cs")
                for nh in range(NT // 2):
                    nc.tensor.matmul(
                        cs[:], ones2[:],
                        e_t[:, 2 * nh:2 * nh + 2, :],
                        start=(nh == 0), stop=(nh == NT // 2 - 1),
                        perf_mode=DR,
                    )
                rb = spool.tile([P, MC], f32, tag="rb", name="rb")
                nc.vector.reciprocal(rb[:], cs[:])
                # PV: OT[e, m] unnormalized (single-bank psum so the next
                # m-chunk's score pairs in ps2 are not blocked behind PV)
                ot_t = otpool.tile([P, ET, MC], f8, tag="ot")
                for eb in range(ET):
                    pso = ps.tile([P, MC], f32, tag="ps", name="pso")
                    for nh in range(NT // 2):
                        nc.tensor.matmul(
                            pso[:],
                            v_t[:, 2 * nh:2 * nh + 2, eb * P:(eb + 1) * P],
                            e_t[:, 2 * nh:2 * nh + 2, :],
                            start=(nh == 0),
                            stop=(nh == NT // 2 - 1),
                            perf_mode=DR,
                        )
                    nc.vector.tensor_copy(ot_t[:, eb, :], pso[:])
                # output linear partial + deferred softmax normalization;
                # all 4 ob blocks collect into one SBUF tile -> single DMA
                res_sb = opool.tile([P, ET, MC], bf16, tag="res")
                for ob in range(ET):
                    psl = ps.tile([P, MC], f32, tag="ps")
                    for eh in range(ET // 2):
                        nc.tensor.matmul(
                            psl[:],
                            wl_t[:, 2 * eh:2 * eh + 2, ob * P:(ob + 1) * P],
                            ot_t[:, 2 * eh:2 * eh + 2, :],
                            start=(eh == 0),
                            stop=(eh == ET // 2 - 1),
                            perf_mode=DR,
                        )
                    nc.vector.tensor_tensor(
                        res_sb[:, ob, :], psl[:], rb[:], mybir.AluOpType.mult
                    )
                nc.sync.dma_start(
                    rs_in[p][mc].rearrange("(o q) m -> q o m", q=P),
                    res_sb[:],
                )

                # ---- reduce-scatter this chunk's partials over the 8 cores;
                # overlaps the next chunk's compute ----
                if not single and not no_cc:
                    nc.gpsimd.collective_compute(
                        "ReduceScatter",
                        mybir.AluOpType.add,
                        ins=[rs_in[p][mc].opt()],
                        outs=[rs_out[p][mc].opt()],
                        replica_groups=[list(range(NCORES))],
                    )
                po_bf = ppool.tile([64, MC], bf16, tag="pobf")
                nc.sync.dma_start(
                    po_bf[:],
                    rs_out[p][mc][:] if not (single or no_cc)
                    else rs_in[p][mc][0:64, :],
                )
                po = ppool.tile([64, MC], f32, tag="po")
                nc.gpsimd.tensor_tensor(
                    po[:], po_bf[:], resid_sb[p][:, mc * MC:(mc + 1) * MC],
                    mybir.AluOpType.add,
                )
                nc.sync.dma_start(out_d[p][:, mc * MC:(mc + 1) * MC], po[:])

    with tile.TileContext(nc) as tc:
        with (
            tc.tile_pool(name="const", bufs=1) as pools["const"],
            tc.tile_pool(name="xt", bufs=1) as pools["xt"],
            tc.tile_pool(name="w", bufs=2) as pools["w"],
            tc.tile_pool(name="qkv", bufs=1) as pools["qkv"],
            tc.tile_pool(name="v", bufs=1) as pools["v"],
            tc.tile_pool(name="expst", bufs=2) as pools["expst"],
            tc.tile_pool(name="ot", bufs=2) as pools["ot"],
            tc.tile_pool(name="small", bufs=2) as pools["small"],
            tc.tile_pool(name="outsb", bufs=2) as pools["outsb"],
            tc.tile_pool(name="post", bufs=2) as pools["post"],
            tc.tile_pool(name="psum", bufs=3, space="PSUM") as pools["psum"],
            tc.tile_pool(name="psum2", bufs=2, space="PSUM") as pools["psum2"],
            tc.tile_pool(name="pscs", bufs=1, space="PSUM") as pools["pscs"],
            tc.tile_pool(name="dram", bufs=1, space="DRAM") as pools["dram"],
        ):
            for _rep in range(reps):
                _emit_rep()

    nc.compile()
    return nc


_NC_CACHE = {}


def _get_nc():
    if "nc" not in _NC_CACHE:
        _NC_CACHE["nc"] = _build()
    return _NC_CACHE["nc"]


def _make_in_maps(inputs):
    f = np.float32
    b8 = ml_dtypes.float8_e4m3

    def c_(x, dt):
        return np.ascontiguousarray(x).astype(dt)

    img = np.asarray(inputs["image_input"], f)
    meta = np.asarray(inputs["metadata_input"], f)
    xt_i = c_(img.T, b8)
    xt_m = c_(meta.T, b8)
    ones2 = np.full((P, 2 * P), ONESVAL, b8)

    # fold v-bias through the output linear into the residual (att rows sum
    # to 1): out += W_lin @ concat_h(bv_h)
    W_li = np.asarray(inputs["W_lin_i"], f)
    W_lm = np.asarray(inputs["W_lin_m"], f)
    corr_i = W_li @ np.asarray(inputs["bv_i"], f).reshape(H * D)
    corr_m = W_lm @ np.asarray(inputs["bv_m"], f).reshape(H * D)
    base_i = np.asarray(inputs["b_lin_i"], f) + corr_i
    base_m = np.asarray(inputs["b_lin_m"], f) + corr_m

    in_maps = []
    for c in range(NCORES):
        sl = slice(64 * c, 64 * (c + 1))
        m = {
            "xt_i": xt_i,
            "xt_m": xt_m,
            "ones2": ones2,
            "resid_i": c_(img[:, sl].T + base_i[sl][:, None], f),
            "resid_m": c_(meta[:, sl].T + base_m[sl][:, None], f),
        }
        for p, (Wq, bq_, Wk, bk_, Wv, Wl) in enumerate([
            (inputs["Wq_m"], inputs["bq_m"], inputs["Wk_i"], inputs["bk_i"],
             inputs["Wv_i"], W_li),
            (inputs["Wq_i"], inputs["bq_i"], inputs["Wk_m"], inputs["bk_m"],
             inputs["Wv_m"], W_lm),
        ]):
            m[f"wq{p}"] = c_(np.asarray(Wq, f)[c].T * SQ, b8)
            m[f"wk{p}"] = c_(np.asarray(Wk, f)[c].T * SK, b8)
            m[f"wv{p}"] = c_(np.asarray(Wv, f)[c].T * SV, b8)
            m[f"wl{p}"] = c_(np.asarray(Wl, f)[:, D * c:D * (c + 1)].T * SL, b8)
            m[f"bq{p}"] = c_(np.asarray(bq_, f)[c].reshape(ET, P).T * SQ, f)
            m[f"bk{p}"] = c_(np.asarray(bk_, f)[c].reshape(ET, P).T * SK, f)
        in_maps.append(m)
    return in_maps


def _assemble(results):
    out_iT = np.concatenate([results[c]["out_i"] for c in range(NCORES)], axis=0)
    out_mT = np.concatenate([results[c]["out_m"] for c in range(NCORES)], axis=0)
    return np.concatenate([out_iT.T, out_mT.T], axis=1).astype(np.float32)


def kernel(**inputs):
    nc = _get_nc()
    in_maps = _make_in_maps(inputs)
    res = run_bass_kernel_spmd(nc, in_maps, list(range(NCORES)))
    return _assemble(res.results)


if __name__ == "__main__":
    _get_nc()
    print("build ok")
